# revision 5
# baseline (speedup 1.0000x reference)
"""Bahdanau additive attention on 8 Trainium2 NeuronCores.

Problem (per reference):
    pq     = query @ Wq.T + bq                         [B, A]
    pk     = einsum("bsk,ak->bsa", keys, Wk) + bk      [B, S, A]
    scores = einsum("bsa,a->bs", tanh(pq[:,None,:] + pk), Ws)
    attn   = softmax(scores, axis=1)                   [B, S]
    context= einsum("bs,bsv->bv", attn, values)        [B, V]
    returns (context, attn)

B=64, S=1024, QD=KD=VD=AD=1024, fp32.

Sharding: data-parallel over batch across 8 cores (8 batches/core),
weights replicated. No collectives.

Per-core kernel layout strategy:
  - All matmuls run in float32r (single-pass fp32, full PE rate).
  - keys arrive [s, k]; PE-transposed (fp32, exact) into keysT [k, s]
    tiles; the PSUM->SBUF copy performs the fp32->f32r conversion.
  - pk.T tiles [a=128, s=512] = WkT @ keysT accumulated over k.
  - tanh fused on ScalarE with per-partition bias = (pq + bq + bk)[a],
    written directly as f32r.
  - scoresT [s=128, 1] = tanh.T @ Ws via N=2 matmuls (fp32r ISA requires
    even moving free dim; Ws duplicated into 2 columns).
  - exp on ScalarE without max-subtraction (|scores| <= ||Ws||_1 <= 32,
    so fp32 exp cannot overflow; softmax is shift-invariant).
  - softmax denominator via ones[128,128].T @ expT -> broadcast to all
    partitions for free; reciprocal on VectorE.
  - context [1, v=512] = attn_normT @ values with values in natural
    [s, v] layout, cast to f32r during the (gpsimd) DMA load.
"""

import sys

if "/opt/trn_rl_repo" not in sys.path:
    sys.path.insert(0, "/opt/trn_rl_repo")

import numpy as np
from contextlib import ExitStack

import concourse.bass as bass
import concourse.tile as tile
from concourse import bacc, mybir
from concourse.bass_utils import run_bass_kernel_spmd
from concourse.masks import make_identity

F32 = mybir.dt.float32
F32R = mybir.dt.float32r
AF = mybir.ActivationFunctionType

NCORES = 8
B, S, D = 64, 1024, 1024  # D = QD = KD = VD = AD
NB = B // NCORES          # local batches per core
P = 128
KC = D // P               # 8 contraction chunks
AT = D // P               # 8 a-tiles
SB = S // P               # 8 s-blocks of 128
SH = S // 512             # 2 s-halves of 512


def _build_nc(repeat=1):
    nc = bacc.Bacc("TRN2", target_bir_lowering=False, debug=False)

    q_d = nc.dram_tensor("query_l", [NB, D], F32, kind="ExternalInput").ap()
    k_d = nc.dram_tensor("keys_l", [NB, S, D], F32, kind="ExternalInput").ap()
    v_d = nc.dram_tensor("values_l", [NB, S, D], F32, kind="ExternalInput").ap()
    wq_d = nc.dram_tensor("Wq", [D, D], F32, kind="ExternalInput").ap()
    wk_d = nc.dram_tensor("Wk", [D, D], F32, kind="ExternalInput").ap()
    bq_d = nc.dram_tensor("bq", [D], F32, kind="ExternalInput").ap()
    bk_d = nc.dram_tensor("bk", [D], F32, kind="ExternalInput").ap()
    ws_d = nc.dram_tensor("Ws", [D], F32, kind="ExternalInput").ap()
    ctx_d = nc.dram_tensor("context_l", [NB, D], F32, kind="ExternalOutput").ap()
    attn_d = nc.dram_tensor("attn_l", [NB, S], F32, kind="ExternalOutput").ap()

    with tile.TileContext(nc) as tc, ExitStack() as ctx:
        const = ctx.enter_context(tc.tile_pool(name="const", bufs=1))
        tpsum = ctx.enter_context(
            tc.tile_pool(name="tpsum", bufs=2, space=bass.MemorySpace.PSUM)
        )
        mpsum = ctx.enter_context(
            tc.tile_pool(name="mpsum", bufs=2, space=bass.MemorySpace.PSUM)
        )
        stpsum = ctx.enter_context(
            tc.tile_pool(name="stpsum", bufs=2, space=bass.MemorySpace.PSUM)
        )
        dcpsum = ctx.enter_context(
            tc.tile_pool(name="dcpsum", bufs=1, space=bass.MemorySpace.PSUM)
        )

        ident = const.tile([P, P], F32, tag="ident")
        make_identity(nc, ident)

        ones_f = const.tile([P, P], F32, tag="ones_f")
        nc.vector.memset(ones_f, 1.0)
        ones_r = const.tile([P, P], F32R, tag="ones_r")
        nc.vector.tensor_copy(ones_r, ones_f)

        # Ws -> [p, at] fp32, then duplicated pairs [p, at, 2] f32r for N=2 matmuls
        ws_f = const.tile([P, AT], F32, tag="ws_f")
        nc.sync.dma_start(ws_f, ws_d.rearrange("(a p) -> p a", p=P))
        ws2_r = const.tile([P, AT, 2], F32R, tag="ws2_r")
        for at in range(AT):
            nc.vector.tensor_copy(ws2_r[:, at, 0:1], ws_f[:, at : at + 1])
            nc.vector.tensor_copy(ws2_r[:, at, 1:2], ws_f[:, at : at + 1])

        # bq + bk -> [p, at] fp32
        bq_f = const.tile([P, AT], F32, tag="bq_f")
        bk_f = const.tile([P, AT], F32, tag="bk_f")
        nc.sync.dma_start(bq_f, bq_d.rearrange("(a p) -> p a", p=P))
        nc.sync.dma_start(bk_f, bk_d.rearrange("(a p) -> p a", p=P))
        bqk_f = const.tile([P, AT], F32, tag="bqk_f")
        nc.vector.tensor_add(bqk_f, bq_f, bk_f)

        # WkT (resident): WkT_all[:, kc, a] = Wk[a, kc*128 + p]
        WkT_all = const.tile([P, KC, D], F32R, tag="WkT_all")
        bias_all = const.tile([P, AT, NB], F32, tag="bias_all")  # pq + bq + bk, [p, at, b]

        with ExitStack() as setup_ctx:
            wstage = setup_ctx.enter_context(tc.tile_pool(name="wstage", bufs=2))
            wqpool = setup_ctx.enter_context(tc.tile_pool(name="wqpool", bufs=1))

            for at in range(AT):
                wk_nat = wstage.tile([P, D], F32, tag="wnat")
                nc.sync.dma_start(wk_nat, wk_d[at * P : (at + 1) * P, :])
                for kc in range(KC):
                    pst = tpsum.tile([P, P], F32, tag="tp")
                    nc.tensor.transpose(pst, wk_nat[:, kc * P : (kc + 1) * P], ident)
                    nc.vector.tensor_copy(
                        WkT_all[:, kc, at * P : (at + 1) * P], pst
                    )

            WqT_all = wqpool.tile([P, KC, D], F32R, tag="WqT_all")
            for at in range(AT):
                wq_nat = wstage.tile([P, D], F32, tag="wnat")
                nc.sync.dma_start(wq_nat, wq_d[at * P : (at + 1) * P, :])
                for qc in range(KC):
                    pst = tpsum.tile([P, P], F32, tag="tp")
                    nc.tensor.transpose(pst, wq_nat[:, qc * P : (qc + 1) * P], ident)
                    nc.vector.tensor_copy(
                        WqT_all[:, qc, at * P : (at + 1) * P], pst
                    )

            # queryT [p(q), qc, b] f32r via PE transposes of query [NB, D]
            q_nat = wqpool.tile([NB, D], F32, tag="q_nat")
            nc.sync.dma_start(q_nat, q_d)
            qT = wqpool.tile([P, KC, NB], F32R, tag="qT")
            for qc in range(KC):
                pst = tpsum.tile([P, NB], F32, tag="tp")
                nc.tensor.transpose(
                    pst, q_nat[0:NB, qc * P : (qc + 1) * P], ident[0:NB, 0:NB]
                )
                nc.vector.tensor_copy(qT[:, qc, :], pst)

            # pqT [a, b] per a-tile; bias_all = pqT + (bq + bk)
            for at in range(AT):
                pqp = tpsum.tile([P, NB], F32, tag="tp")
                for qc in range(KC):
                    nc.tensor.matmul(
                        pqp,
                        WqT_all[:, qc, at * P : (at + 1) * P],
                        qT[:, qc, :],
                        start=(qc == 0),
                        stop=(qc == KC - 1),
                    )
                nc.vector.tensor_scalar_add(
                    bias_all[:, at, :], pqp, bqk_f[:, at : at + 1]
                )

        # ---- main loop over local batches ----
        kpool = ctx.enter_context(tc.tile_pool(name="kpool", bufs=3))
        ktpool = ctx.enter_context(tc.tile_pool(name="ktpool", bufs=2))
        thpool = ctx.enter_context(tc.tile_pool(name="thpool", bufs=12))
        vpool = ctx.enter_context(tc.tile_pool(name="vpool", bufs=12))
        smpool = ctx.enter_context(tc.tile_pool(name="smpool", bufs=3))
        outpool = ctx.enter_context(tc.tile_pool(name="outpool", bufs=4))

        rep_ctx = ExitStack()
        if repeat > 1:
            rep_ctx.enter_context(tc.For_i(0, repeat, 1))

        for b in range(NB):
            # values prefetch (cast to f32r on the way in via SWDGE)
            vals = []
            for sb in range(SB):
                vt = vpool.tile([P, D], F32R, tag="vals")
                nc.gpsimd.dma_start(vt, v_d[b, sb * P : (sb + 1) * P, :])
                vals.append(vt)

            expT = smpool.tile([P, SB], F32R, tag="expT")

            for sh in range(SH):
                kt = ktpool.tile([P, KC, 512], F32R, tag="kt")
                for sb4 in range(4):
                    s0 = sh * 512 + sb4 * P
                    knat = kpool.tile([P, D], F32, tag="knat")
                    nc.sync.dma_start(knat, k_d[b, s0 : s0 + P, :])
                    for kc in range(KC):
                        pst = tpsum.tile([P, P], F32, tag="tp")
                        nc.tensor.transpose(
                            pst, knat[:, kc * P : (kc + 1) * P], ident
                        )
                        nc.vector.tensor_copy(
                            kt[:, kc, sb4 * P : (sb4 + 1) * P], pst
                        )

                th_tiles = []
                for at in range(AT):
                    mp = mpsum.tile([P, 512], F32, tag="mp")
                    for kc in range(KC):
                        nc.tensor.matmul(
                            mp,
                            WkT_all[:, kc, at * P : (at + 1) * P],
                            kt[:, kc, :],
                            start=(kc == 0),
                            stop=(kc == KC - 1),
                        )
                    th = thpool.tile([P, 512], F32R, tag="th")
                    nc.scalar.activation(
                        th, mp, AF.Tanh, bias=bias_all[:, at, b : b + 1]
                    )
                    th_tiles.append(th)

                for sb4 in range(4):
                    j = sh * 4 + sb4
                    stp = stpsum.tile([P, 2], F32, tag="stp")
                    for at in range(AT):
                        nc.tensor.matmul(
                            stp,
                            th_tiles[at][:, sb4 * P : (sb4 + 1) * P],
                            ws2_r[:, at, :],
                            start=(at == 0),
                            stop=(at == AT - 1),
                        )
                    nc.scalar.activation(expT[:, j : j + 1], stp[:, 0:1], AF.Exp)

            # softmax denominator, broadcast to all partitions via ones-matmul
            dps = dcpsum.tile([P, SB], F32, tag="dps")
            nc.tensor.matmul(dps, ones_r, expT, start=True, stop=True)
            den = smpool.tile([P, 1], F32, tag="den")
            nc.vector.reduce_sum(den, dps, axis=mybir.AxisListType.X)
            rden = smpool.tile([P, 1], F32, tag="rden")
            nc.vector.reciprocal(rden, den)

            attn_r = smpool.tile([P, SB], F32R, tag="attn_r")
            nc.vector.tensor_scalar_mul(attn_r, expT, rden)
            attn_f = outpool.tile([P, SB], F32, tag="attn_f")
            nc.vector.tensor_scalar_mul(attn_f, expT, rden)
            nc.sync.dma_start(
                attn_d[b, :].rearrange("(sb p) -> p sb", p=P), attn_f
            )

            # context
            for vc in range(2):
                cps = dcpsum.tile([1, 512], F32, tag="cps")
                for sb in range(SB):
                    nc.tensor.matmul(
                        cps,
                        attn_r[:, sb : sb + 1],
                        vals[sb][:, vc * 512 : (vc + 1) * 512],
                        start=(sb == 0),
                        stop=(sb == SB - 1),
                    )
                ctx_sb = outpool.tile([1, 512], F32, tag="ctx_sb")
                nc.vector.tensor_copy(ctx_sb, cps)
                nc.sync.dma_start(ctx_d[b, vc * 512 : (vc + 1) * 512], ctx_sb)

        rep_ctx.close()

    nc.compile()
    if not nc.is_finalized():
        nc.finalize()
    return nc


_NC_CACHE = None


def _get_nc():
    global _NC_CACHE
    if _NC_CACHE is None:
        _NC_CACHE = _build_nc()
    return _NC_CACHE


def kernel(query, keys, values, Wq, bq, Wk, bk, Ws, **kw):
    query = np.ascontiguousarray(np.asarray(query, dtype=np.float32))
    keys = np.ascontiguousarray(np.asarray(keys, dtype=np.float32))
    values = np.ascontiguousarray(np.asarray(values, dtype=np.float32))
    Wq = np.ascontiguousarray(np.asarray(Wq, dtype=np.float32))
    Wk = np.ascontiguousarray(np.asarray(Wk, dtype=np.float32))
    bq = np.ascontiguousarray(np.asarray(bq, dtype=np.float32))
    bk = np.ascontiguousarray(np.asarray(bk, dtype=np.float32))
    Ws = np.ascontiguousarray(np.asarray(Ws, dtype=np.float32))

    nc = _get_nc()
    in_maps = []
    for c in range(NCORES):
        lo, hi = c * NB, (c + 1) * NB
        in_maps.append(
            {
                "query_l": query[lo:hi],
                "keys_l": keys[lo:hi],
                "values_l": values[lo:hi],
                "Wq": Wq,
                "Wk": Wk,
                "bq": bq,
                "bk": bk,
                "Ws": Ws,
            }
        )
    res = run_bass_kernel_spmd(nc, in_maps, core_ids=list(range(NCORES)))
    context = np.concatenate([r["context_l"] for r in res.results], axis=0)
    attn = np.concatenate([r["attn_l"] for r in res.results], axis=0)
    return context, attn
